# revision 60
# baseline (speedup 1.0000x reference)
"""Trainium2 Bass kernel for nn_LocalMHA (block-diagonal windowed MHA).

Contract: kernel(**inputs) takes FULL unsharded inputs as produced by
reference.setup_inputs() and returns the FULL output [B, T, D] fp32.

Sharding: data-parallel over flattened (B*T) tokens across 8 cores
(8192 tokens/core; 64-token windows never cross a core boundary).
Weights/tables replicated.

Per-core pipeline over tiles of 128 tokens (= 2 windows), software-
pipelined in 3 stages (iteration k issues A(k), B(k-1), C(k-2)).

All transposes run on the DMA engines (XBAR dma_start_transpose on
bf16), which removes every PE transpose AND the ACT psum->SBUF copy
that followed each one:

  A: LN stats (DVE bn_stats/aggr) -> rsqrt via Newton on Pool ->
     xn=(x-m)*rstd to bf16 (DVE) -> xnT via DMA transpose -> QKV
     matmuls (bf16, f32 psum) -> ACT copies q,k,v psum->SBUF bf16 ->
     RoPE mults on DVE (bf16 SBUF, 2x mode), adds on Pool, writing
     q'/k' into one [128,2,512] tile.
  B: qkT via one DMA transpose -> QK per (head, window) into ONE psum
     bank [128,8,64] (valid blocks only) -> 2 dense exp ops (ACT) into
     pre-zeroed probs [128,8,128] bf16 -> row-sums (DVE reduce over
     zeros-padded rows) -> reciprocal -> 8 per-head normalize TSPs
     (DVE 4x mode).
  C: pT via DMA transpose -> PV per head (K=128 over the zero-padded
     block-diagonal probs^T) -> attnT copy to SBUF bf16 (ACT) -> WO
     matmuls -> residual add -> store.

bf16 interior with f32 psum accumulation everywhere.
"""

import hashlib
import os
import sys

import numpy as np

if "/opt/trn_rl_repo" not in sys.path:
    sys.path.insert(0, "/opt/trn_rl_repo")

import concourse.bass as bass  # noqa: E402
import concourse.bacc as bacc  # noqa: E402
import concourse.tile as tile  # noqa: E402
from concourse import mybir  # noqa: E402
from concourse import bass_utils  # noqa: E402

N_CORES = 8
B, T, D = 4, 16384, 512
H, HD = 8, 64
WINDOWS = 256
TOK_PER_CORE = B * T // N_CORES  # 8192
P = 128  # tokens per tile (2 windows)
EPS = 1e-6

F32 = mybir.dt.float32
BF16 = mybir.dt.bfloat16
ALU = mybir.AluOpType
ACTF = mybir.ActivationFunctionType

_build_cache = {}

# engine-assignment knobs, tunable without touching the kernel body
CFG = dict(
    newton_on="pool", newton_iters=1,     # rsqrt Newton chain: pool | dve
    xn_on="dve",          # (x-m)*rstd apply: dve | act
    adds_on="pool",       # rope tm+ts adds: pool | dve
    v_copy_on="act",      # v psum->sbuf copy: act | pool
    attn_copy_on="act",   # attnT psum->sbuf copy: act | pool | dve
    resid_on="pool",      # residual add: pool | dve
    group=1,
    nprobs=4,
    x_bufs=13,
    lg_b=2, qkv_b=1, xa_b=1, y_b=1, xnT_b=1,
    xnT_copy_on="pool",   # xnT psum->sbuf copy: pool | act | dve
    o_load=0, o_ln=1, o_xnT=2, o_qkv=3, o_rope=4, o_qkT=5, o_qk=6,
    o_softmax=7, o_pT=8, o_pv=10, o_wo=11,
    issue_order=("softmax", "qk", "pv", "wo", "rope", "qkT", "pT",
                 "qkv", "xnT", "ln", "load"),
)


def _marker_shape():
    with open(os.path.abspath(__file__), "rb") as f:
        h = int.from_bytes(hashlib.sha256(f.read()).digest()[:8], "little")
    return [1 + h % 1021, 1 + (h // 1021) % 1021]


def _rot_view(t):
    """View of t [128, (h=8, d=64)] with halves of each head's d swapped:
    elem (p, h, b, r) -> t[p, h*64 + (1-b)*32 + r]."""
    v = t[:, 32:]
    return bass.AP(tensor=v.tensor, offset=v.offset,
                   ap=[list(v.ap[0])] + [[64, 8], [-32, 2], [1, 32]])


def _rot_view2(t):
    """Rot-half view over the q and k blocks of t [128, 3, (h d)]:
    elem (p, b, h, half, r) -> t[p, b, h*64 + (1-half)*32 + r]."""
    v = t[:, 0, 32:]
    return bass.AP(tensor=v.tensor, offset=v.offset,
                   ap=[list(v.ap[0])] + [[D, 2], [64, 8], [-32, 2],
                                         [1, 32]])


def build(n_tokens=TOK_PER_CORE, with_bias_row=False, cfg=None):
    cfg = dict(CFG, **(cfg or {}))
    nc = bacc.Bacc("TRN2", target_bir_lowering=False, debug=False,
                   num_devices=N_CORES)
    x_in = nc.dram_tensor("x", [n_tokens, D], F32,
                          kind="ExternalInput").ap()
    wqkv = nc.dram_tensor("wqkv", [D, 3 * D], BF16,
                          kind="ExternalInput").ap()
    wo_in = nc.dram_tensor("wo", [D, D], BF16, kind="ExternalInput").ap()
    cos_in = nc.dram_tensor("cos", [P, D], BF16,
                            kind="ExternalInput").ap()
    ssin_in = nc.dram_tensor("ssin", [P, D], BF16,
                             kind="ExternalInput").ap()
    ident_in = nc.dram_tensor("ident", [P, P], BF16,
                              kind="ExternalInput").ap()
    if with_bias_row:
        brow_in = nc.dram_tensor("brow", [1, 3 * D], BF16,
                                 kind="ExternalInput").ap()
        ones_in = nc.dram_tensor("onesrow", [1, P], BF16,
                                 kind="ExternalInput").ap()
    y_out = nc.dram_tensor("y", [n_tokens, D], F32,
                           kind="ExternalOutput").ap()

    n_tiles = n_tokens // P

    def eng(name):
        return {"pool": nc.gpsimd, "dve": nc.vector,
                "act": nc.scalar}[cfg[name]]

    with tile.TileContext(nc) as tc:
        with (
            tc.tile_pool(name="const", bufs=1) as const,
            tc.tile_pool(name="io", bufs=4) as io,
            tc.tile_pool(name="work", bufs=3) as work,
            tc.tile_pool(name="small", bufs=4) as small,
            tc.tile_pool(name="ps_qkv", bufs=cfg["qkv_b"],
                         space="PSUM") as ps_qkv,
            tc.tile_pool(name="ps_lg", bufs=cfg["lg_b"],
                         space="PSUM") as ps_lg,
            tc.tile_pool(name="ps_xa", bufs=cfg["xa_b"],
                         space="PSUM") as ps_xa,
            tc.tile_pool(name="ps_y", bufs=cfg["y_b"],
                         space="PSUM") as ps_y,
            tc.tile_pool(name="ps_xnT", bufs=cfg["xnT_b"],
                         space="PSUM") as ps_xnT,
        ):
            # ---- constants (bf16 prepared host-side) ----
            wqkv_sb = const.tile([P, 4, 3 * D], BF16)
            nc.gpsimd.dma_start(
                out=wqkv_sb, in_=wqkv.rearrange("(c p) n -> p c n", p=P))
            wo_sb = const.tile([P, 4, D], BF16)
            nc.gpsimd.dma_start(
                out=wo_sb, in_=wo_in.rearrange("(c p) n -> p c n", p=P))
            cos2_sb = const.tile([P, 2, D], BF16)
            nc.sync.dma_start(out=cos2_sb[:, 0, :], in_=cos_in)
            nc.sync.dma_start(out=cos2_sb[:, 1, :], in_=cos_in)
            ssin2_sb = const.tile([P, 2, D], BF16)
            nc.sync.dma_start(out=ssin2_sb[:, 0, :], in_=ssin_in)
            nc.sync.dma_start(out=ssin2_sb[:, 1, :], in_=ssin_in)
            ident_bf = const.tile([P, P], BF16)
            nc.gpsimd.dma_start(out=ident_bf, in_=ident_in)
            if with_bias_row:
                brow_sb = const.tile([1, 3 * D], BF16)
                nc.gpsimd.dma_start(out=brow_sb, in_=brow_in)
                ones_sb = const.tile([1, P], BF16)
                nc.gpsimd.dma_start(out=ones_sb, in_=ones_in)

            # persistent probs tiles, pre-zeroed once: exp/normalize only
            # touch valid blocks/rows, so cross-window slots stay 0 and
            # the K=128 PV contracts over an exact block-diagonal.
            nprobs = cfg["nprobs"]
            probs_tiles = []
            for z in range(nprobs):
                pz = work.tile([P, H, P], BF16, tag="probs",
                               name=f"pz{z}", bufs=nprobs)
                nc.gpsimd.memset(pz, 0.0)
                probs_tiles.append(pz)

            st = {}

            def stage_load(i):
                t0 = i * P
                x_t = io.tile([P, D], F32, tag="x", name=f"x{i}",
                              bufs=cfg["x_bufs"])
                nc.sync.dma_start(out=x_t, in_=x_in[t0:t0 + P, :])
                st[i] = {"x": x_t}

            def stage_ln(i):
                s = st[i]
                x_t = s["x"]
                stats = small.tile([P, 6], F32, tag="st", name=f"st{i}")
                nc.vector.bn_stats(out=stats, in_=x_t)
                mv = small.tile([P, 2], F32, tag="mv", name=f"mv{i}")
                nc.vector.bn_aggr(out=mv, in_=stats)
                # rsqrt(var+eps) via Newton on [P,1] tiles: y0=1.5-0.5v,
                # 2 iterations; v is within ~[0.6, 1.5] for unit-variance
                # rows so the result is accurate to ~1e-4.
                ne = eng("newton_on")
                v_t = small.tile([P, 1], F32, tag="vv", name=f"vv{i}")
                ne.tensor_scalar(
                    out=v_t, in0=mv[:, 1:2], scalar1=EPS, scalar2=None,
                    op0=ALU.add, op1=ALU.bypass)
                rstd = small.tile([P, 1], F32, tag="rs", name=f"rs{i}")
                ne.tensor_scalar(
                    out=rstd, in0=v_t, scalar1=-0.5, scalar2=1.5,
                    op0=ALU.mult, op1=ALU.add)
                tnv = small.tile([P, 1], F32, tag="tn", name=f"tn{i}")
                for _ in range(cfg["newton_iters"]):
                    ne.tensor_tensor(out=tnv, in0=rstd, in1=rstd,
                                     op=ALU.mult)
                    ne.tensor_tensor(out=tnv, in0=tnv, in1=v_t,
                                     op=ALU.mult)
                    ne.tensor_scalar(
                        out=tnv, in0=tnv, scalar1=-0.5, scalar2=1.5,
                        op0=ALU.mult, op1=ALU.add)
                    ne.tensor_tensor(out=rstd, in0=rstd, in1=tnv,
                                     op=ALU.mult)
                xn = work.tile([P, D], BF16, tag="xn", name=f"xn{i}",
                               bufs=6)
                if cfg["xn_on"] == "dve":
                    nc.vector.tensor_scalar(
                        out=xn, in0=x_t, scalar1=mv[:, 0:1], scalar2=rstd,
                        op0=ALU.subtract, op1=ALU.mult)
                else:
                    # xn = x*rstd + (-m*rstd) on ACT
                    nmr = small.tile([P, 1], F32, tag="nm", name=f"nm{i}")
                    ne.tensor_tensor(out=nmr, in0=mv[:, 0:1], in1=rstd,
                                     op=ALU.mult)
                    ne.tensor_scalar(out=nmr, in0=nmr, scalar1=-1.0,
                                     scalar2=None, op0=ALU.mult,
                                     op1=ALU.bypass)
                    nc.scalar.activation(out=xn, in_=x_t, func=ACTF.Copy,
                                         scale=rstd, bias=nmr)

                s["xn"] = xn

            def stage_xnT(i):
                s = st[i]
                xn = s.pop("xn")
                xnT_ps = ps_xnT.tile([P, 4, P], BF16, tag="xnTp",
                                     name=f"xnTp{i}")
                for c in range(4):
                    nc.tensor.transpose(
                        xnT_ps[:, c, :], xn[:, c * P:(c + 1) * P],
                        ident_bf)
                xnT = work.tile([P, 4, P], BF16, tag="xnT",
                                name=f"xnT{i}", bufs=6)
                ce = cfg["xnT_copy_on"]
                if ce == "act":
                    nc.scalar.copy(out=xnT, in_=xnT_ps)
                else:
                    eng("xnT_copy_on").tensor_copy(out=xnT, in_=xnT_ps)
                s["xnT"] = xnT

            def stage_qkv(i):
                s = st[i]
                xnT = s["xnT"]
                qkv_ps = ps_qkv.tile([P, 3, D], F32, tag="qkv",
                                     name=f"qkv{i}")
                for bk in range(3):
                    pt = qkv_ps[:, bk, :]
                    for c in range(4):
                        nc.tensor.matmul(
                            pt, xnT[:, c, :],
                            wqkv_sb[:, c, bk * D:(bk + 1) * D],
                            start=(c == 0),
                            stop=(c == 3 and not with_bias_row))
                    if with_bias_row:
                        nc.tensor.matmul(
                            pt, ones_sb, brow_sb[:, bk * D:(bk + 1) * D],
                            start=False, stop=True)
                s["qkv_ps"] = qkv_ps

            def stage_rope(i):
                s = st[i]
                qkv_ps = s.pop("qkv_ps")
                # ONE psum->SBUF bf16 copy for q,k,v (frees all 3 banks
                # with a single ordinal; v is consumed in place by PV)
                qkvc = work.tile([P, 3, D], BF16, tag="qkvc",
                                 name=f"qkvc{i}", bufs=10)
                if cfg["v_copy_on"] == "act":
                    nc.scalar.copy(out=qkvc, in_=qkv_ps)
                else:
                    nc.scalar.copy(out=qkvc[:, 0:2, :],
                                   in_=qkv_ps[:, 0:2, :])
                    eng("v_copy_on").tensor_copy(out=qkvc[:, 2, :],
                                                 in_=qkv_ps[:, 2, :])

                # RoPE on q and k in single wide ops
                qk2 = work.tile([P, 2, D], BF16, tag="qk2",
                                name=f"qk2{i}", bufs=6)
                tm2 = work.tile([P, 2, D], BF16, tag="tm2",
                                name=f"tm2_{i}")
                nc.vector.tensor_tensor(out=tm2, in0=qkvc[:, 0:2, :],
                                        in1=cos2_sb, op=ALU.mult)
                ts2 = work.tile([P, 2, D], BF16, tag="ts2",
                                name=f"ts2_{i}")
                qk_rot = _rot_view2(qkvc)
                nc.vector.tensor_tensor(
                    out=ts2.rearrange("p b (h t r) -> p b h t r", h=8,
                                      t=2),
                    in0=qk_rot,
                    in1=ssin2_sb.rearrange("p b (h t r) -> p b h t r",
                                           h=8, t=2),
                    op=ALU.mult)
                eng("adds_on").tensor_tensor(out=qk2, in0=tm2, in1=ts2,
                                             op=ALU.add)
                s["v"] = qkvc
                s["qk2"] = qk2

            def stage_qkT(i):
                s = st[i]
                qkT = work.tile([P, 2 * H, P], BF16, tag="qkT",
                                name=f"qkT{i}", bufs=6)
                nc.sync.dma_start_transpose(
                    out=qkT,
                    in_=s.pop("qk2").rearrange("p a b -> p (a b)"))
                s["qkT"] = qkT

            def stage_qk(i):
                s = st[i]
                qkT = s["qkT"]

                def qT_h(h):
                    return qkT[(h % 2) * 64:(h % 2) * 64 + 64, h // 2, :]

                def kT_h(h):
                    return qkT[(h % 2) * 64:(h % 2) * 64 + 64,
                               4 + h // 2, :]

                # QK per (head, window): all-valid logits in ONE psum
                # bank [128, 8, 64] (window w on partitions w*64..).
                lg = ps_lg.tile([P, H, 64], F32, tag="lg",
                                name=f"lg{i}")
                for h in range(H):
                    qT = qT_h(h)
                    kT = kT_h(h)
                    for w in range(2):
                        sl = slice(w * 64, w * 64 + 64)
                        nc.tensor.matmul(lg[sl, h, :], qT[:, sl],
                                         kT[:, sl], start=True,
                                         stop=True)
                s["lg"] = lg

            def stage_softmax(i):
                s = st[i]
                lg = s.pop("lg")
                probs = probs_tiles[i % nprobs]
                # dense exp of the valid diagonal blocks (all heads in
                # one op per window)
                nc.scalar.activation(
                    out=probs[0:64, :, 0:64], in_=lg[0:64, :, :],
                    func=ACTF.Exp, scale=0.125)
                nc.scalar.activation(
                    out=probs[64:128, :, 64:128], in_=lg[64:128, :, :],
                    func=ACTF.Exp, scale=0.125)
                # row sums over the zero-padded full rows (zeros add 0)
                sums = small.tile([P, H], F32, tag="sm", name=f"sm{i}")
                nc.vector.tensor_reduce(
                    out=sums, in_=probs, axis=mybir.AxisListType.X,
                    op=ALU.add)
                nc.vector.reciprocal(out=sums, in_=sums)
                for h in range(H):
                    nc.vector.tensor_scalar_mul(
                        out=probs[:, h, :], in0=probs[:, h, :],
                        scalar1=sums[:, h:h + 1])
                s["probs"] = probs

            def stage_pT(i):
                s = st[i]
                pT = work.tile([P, H, P], BF16, tag="pT", name=f"pT{i}",
                               bufs=6)
                nc.sync.dma_start_transpose(
                    out=pT,
                    in_=s.pop("probs").rearrange("p a b -> p (a b)"))
                s["pT"] = pT

            def stage_pv(i):
                s = st[i]
                pT = s["pT"]

                attnT_ps = ps_xa.tile([P, 4, P], F32, tag="xa",
                                      name=f"aTp{i}")
                for idx in range(H):
                    h = 2 * (idx % 4) + idx // 4
                    nc.tensor.matmul(
                        attnT_ps[(h % 2) * 64:(h % 2) * 64 + 64,
                                 h // 2, :],
                        s["v"][:, 2, h * 64:(h + 1) * 64], pT[:, h, :],
                        start=True, stop=True)
                attnT = work.tile([P, 4, P], BF16, tag="attnT",
                                  name=f"aT{i}", bufs=6)
                ce = cfg["attn_copy_on"]
                if ce == "act":
                    nc.scalar.copy(out=attnT, in_=attnT_ps)
                else:
                    eng("attn_copy_on").tensor_copy(out=attnT,
                                                    in_=attnT_ps)
                s["attnT"] = attnT
                if cfg["o_wo"] is None:
                    stage_wo(i)

            def stage_wo(i):
                s = st.pop(i)
                attnT = s["attnT"]
                y_ps = ps_y.tile([P, D], F32, tag="y", name=f"y{i}")
                for c in range(4):
                    nc.tensor.matmul(y_ps, attnT[:, c, :],
                                     wo_sb[:, c, :],
                                     start=(c == 0), stop=(c == 3))
                o_t = io.tile([P, D], F32, tag="o", name=f"o{i}")
                eng("resid_on").tensor_tensor(out=o_t, in0=y_ps,
                                              in1=s["x"], op=ALU.add)
                t0 = i * P
                nc.sync.dma_start(out=y_out[t0:t0 + P, :], in_=o_t)

            # (offset, stage) in per-iteration ISSUE order: work whose
            # deps completed in earlier iterations is issued first so
            # each in-order engine stream leads with ready work.
            by_name = {
                "load": (cfg["o_load"], stage_load),
                "ln": (cfg["o_ln"], stage_ln),
                "xnT": (cfg["o_xnT"], stage_xnT),
                "qkv": (cfg["o_qkv"], stage_qkv),
                "rope": (cfg["o_rope"], stage_rope),
                "qkT": (cfg["o_qkT"], stage_qkT),
                "qk": (cfg["o_qk"], stage_qk),
                "softmax": (cfg["o_softmax"], stage_softmax),
                "pT": (cfg["o_pT"], stage_pT),
                "pv": (cfg["o_pv"], stage_pv),
            }
            if cfg["o_wo"] is not None:
                by_name["wo"] = (cfg["o_wo"], stage_wo)
            stages = [by_name[n] for n in cfg["issue_order"]
                      if n in by_name]
            assert len(stages) == len(by_name)
            depth = max(o for o, _ in stages) + 1
            G = cfg["group"]
            n_groups = n_tiles // G
            for k in range(n_groups + depth - 1):
                for o, fn in stages:
                    g = k - o
                    if 0 <= g < n_groups:
                        for t in range(G * g, G * g + G):
                            fn(t)

    mk_shape = _marker_shape()
    nc.dram_tensor("uniq_marker", mk_shape, F32, kind="ExternalInput")
    nc.compile()
    return nc, tuple(mk_shape)


def _host_prep(inputs):
    x = np.asarray(inputs["x"], np.float32)
    ln_scale = np.asarray(inputs["ln_scale"], np.float32)
    ln_bias = np.asarray(inputs["ln_bias"], np.float32)
    wq = np.asarray(inputs["wq"], np.float32).reshape(D, D)
    wk = np.asarray(inputs["wk"], np.float32).reshape(D, D)
    wv = np.asarray(inputs["wv"], np.float32).reshape(D, D)
    wo = np.asarray(inputs["wo"], np.float32)
    windows = int(np.asarray(inputs["windows"]))
    assert windows == WINDOWS, f"unsupported windows={windows}"
    assert x.shape == (B, T, D)

    import ml_dtypes
    bf16 = ml_dtypes.bfloat16
    wcat = np.concatenate([wq, wk, wv], axis=1)  # [D, 3D]
    wqkv = np.ascontiguousarray(wcat * ln_scale[:, None]).astype(bf16)
    has_bias = bool(np.any(ln_bias != 0))
    brow = (ln_bias @ wcat).reshape(1, 3 * D).astype(bf16)

    n = T // windows  # 64
    inv = (1.0 / 10000.0 ** (np.arange(0, HD, 2, dtype=np.float64) / HD))
    pos = np.arange(n, dtype=np.float64)
    f = pos[:, None] * inv[None, :]  # [64, 32]
    ang = np.concatenate([f, f], axis=1)  # [64, 64]
    cos1 = np.cos(ang)
    sin1 = np.sin(ang)
    ssin1 = sin1.copy()
    ssin1[:, 0:32] *= -1.0  # sign folded: rot contribution
    cos_t = np.tile(np.tile(cos1, (2, 1)), (1, H)).astype(bf16)
    ssin_t = np.tile(np.tile(ssin1, (2, 1)), (1, H)).astype(bf16)
    return (x.reshape(B * T, D), wqkv, wo.astype(bf16), cos_t, ssin_t,
            has_bias, brow)


def kernel(**inputs):
    (xf, wqkv, wo, cos_t, ssin_t, has_bias, brow) = _host_prep(inputs)

    key = ("full", has_bias)
    if key not in _build_cache:
        _build_cache[key] = build(TOK_PER_CORE, with_bias_row=has_bias)
    nc, mk_shape = _build_cache[key]

    import ml_dtypes
    shared = {
        "wqkv": wqkv, "wo": np.ascontiguousarray(wo),
        "cos": cos_t, "ssin": ssin_t,
        "ident": np.eye(P, dtype=ml_dtypes.bfloat16),
        "uniq_marker": np.zeros(mk_shape, np.float32),
    }
    if has_bias:
        import ml_dtypes
        shared["brow"] = brow
        shared["onesrow"] = np.ones((1, P), ml_dtypes.bfloat16)
    in_maps = []
    for c in range(N_CORES):
        m = dict(shared)
        m["x"] = np.ascontiguousarray(
            xf[c * TOK_PER_CORE:(c + 1) * TOK_PER_CORE])
        in_maps.append(m)

    res = bass_utils.run_bass_kernel_spmd(
        nc, in_maps, core_ids=list(range(N_CORES)))
    out = np.concatenate([res.results[c]["y"] for c in range(N_CORES)],
                         axis=0)
    return out.reshape(B, T, D)


# revision 62
# speedup vs baseline: 1.0426x; 1.0426x over previous
"""Trainium2 Bass kernel for nn_LocalMHA (block-diagonal windowed MHA).

Contract: kernel(**inputs) takes FULL unsharded inputs as produced by
reference.setup_inputs() and returns the FULL output [B, T, D] fp32.

Sharding: data-parallel over flattened (B*T) tokens across 8 cores
(8192 tokens/core; 64-token windows never cross a core boundary).
Weights/tables replicated.

Per-core pipeline over tiles of 128 tokens (= 2 windows), software-
pipelined in 3 stages (iteration k issues A(k), B(k-1), C(k-2)).

All transposes run on the DMA engines (XBAR dma_start_transpose on
bf16), which removes every PE transpose AND the ACT psum->SBUF copy
that followed each one:

  A: LN stats (DVE bn_stats/aggr) -> rsqrt via Newton on Pool ->
     xn=(x-m)*rstd to bf16 (DVE) -> xnT via DMA transpose -> QKV
     matmuls (bf16, f32 psum) -> ACT copies q,k,v psum->SBUF bf16 ->
     RoPE mults on DVE (bf16 SBUF, 2x mode), adds on Pool, writing
     q'/k' into one [128,2,512] tile.
  B: qkT via one DMA transpose -> QK per (head, window) into ONE psum
     bank [128,8,64] (valid blocks only) -> 2 dense exp ops (ACT) into
     pre-zeroed probs [128,8,128] bf16 -> row-sums (DVE reduce over
     zeros-padded rows) -> reciprocal -> 8 per-head normalize TSPs
     (DVE 4x mode).
  C: pT via DMA transpose -> PV per head (K=128 over the zero-padded
     block-diagonal probs^T) -> attnT copy to SBUF bf16 (ACT) -> WO
     matmuls -> residual add -> store.

bf16 interior with f32 psum accumulation everywhere.
"""

import hashlib
import os
import sys

import numpy as np

if "/opt/trn_rl_repo" not in sys.path:
    sys.path.insert(0, "/opt/trn_rl_repo")

import concourse.bass as bass  # noqa: E402
import concourse.bacc as bacc  # noqa: E402
import concourse.tile as tile  # noqa: E402
from concourse import mybir  # noqa: E402
from concourse import bass_utils  # noqa: E402

N_CORES = 8
B, T, D = 4, 16384, 512
H, HD = 8, 64
WINDOWS = 256
TOK_PER_CORE = B * T // N_CORES  # 8192
P = 128  # tokens per tile (2 windows)
EPS = 1e-6

F32 = mybir.dt.float32
BF16 = mybir.dt.bfloat16
ALU = mybir.AluOpType
ACTF = mybir.ActivationFunctionType

_build_cache = {}

# engine-assignment knobs, tunable without touching the kernel body
CFG = dict(
    newton_on="pool", newton_iters=1, sums_via="tsp",     # rsqrt Newton chain: pool | dve
    xn_on="dve",          # (x-m)*rstd apply: dve | act
    adds_on="pool",       # rope tm+ts adds: pool | dve
    v_copy_on="act",      # v psum->sbuf copy: act | pool
    attn_copy_on="act",   # attnT psum->sbuf copy: act | pool | dve
    resid_on="pool",      # residual add: pool | dve
    group=1,
    nprobs=4,
    x_bufs=13,
    lg_b=2, qkv_b=1, xa_b=1, y_b=1, xnT_b=1,
    xnT_copy_on="pool",   # xnT psum->sbuf copy: pool | act | dve
    o_load=0, o_ln=1, o_xnT=2, o_qkv=3, o_rope=4, o_qkT=5, o_qk=6,
    o_softmax=7, o_pT=8, o_pv=10, o_wo=11,
    issue_order=("softmax", "qk", "pv", "wo", "rope", "qkT", "pT",
                 "qkv", "xnT", "ln", "load"),
)


def _marker_shape():
    with open(os.path.abspath(__file__), "rb") as f:
        h = int.from_bytes(hashlib.sha256(f.read()).digest()[:8], "little")
    return [1 + h % 1021, 1 + (h // 1021) % 1021]


def _rot_view(t):
    """View of t [128, (h=8, d=64)] with halves of each head's d swapped:
    elem (p, h, b, r) -> t[p, h*64 + (1-b)*32 + r]."""
    v = t[:, 32:]
    return bass.AP(tensor=v.tensor, offset=v.offset,
                   ap=[list(v.ap[0])] + [[64, 8], [-32, 2], [1, 32]])


def _rot_view2(t):
    """Rot-half view over the q and k blocks of t [128, 3, (h d)]:
    elem (p, b, h, half, r) -> t[p, b, h*64 + (1-half)*32 + r]."""
    v = t[:, 0, 32:]
    return bass.AP(tensor=v.tensor, offset=v.offset,
                   ap=[list(v.ap[0])] + [[D, 2], [64, 8], [-32, 2],
                                         [1, 32]])


def build(n_tokens=TOK_PER_CORE, with_bias_row=False, cfg=None):
    cfg = dict(CFG, **(cfg or {}))
    nc = bacc.Bacc("TRN2", target_bir_lowering=False, debug=False,
                   num_devices=N_CORES)
    x_in = nc.dram_tensor("x", [n_tokens, D], F32,
                          kind="ExternalInput").ap()
    wqkv = nc.dram_tensor("wqkv", [D, 3 * D], BF16,
                          kind="ExternalInput").ap()
    wo_in = nc.dram_tensor("wo", [D, D], BF16, kind="ExternalInput").ap()
    cos_in = nc.dram_tensor("cos", [P, D], BF16,
                            kind="ExternalInput").ap()
    ssin_in = nc.dram_tensor("ssin", [P, D], BF16,
                             kind="ExternalInput").ap()
    ident_in = nc.dram_tensor("ident", [P, P], BF16,
                              kind="ExternalInput").ap()
    if with_bias_row:
        brow_in = nc.dram_tensor("brow", [1, 3 * D], BF16,
                                 kind="ExternalInput").ap()
        ones_in = nc.dram_tensor("onesrow", [1, P], BF16,
                                 kind="ExternalInput").ap()
    y_out = nc.dram_tensor("y", [n_tokens, D], F32,
                           kind="ExternalOutput").ap()

    n_tiles = n_tokens // P

    def eng(name):
        return {"pool": nc.gpsimd, "dve": nc.vector,
                "act": nc.scalar}[cfg[name]]

    with tile.TileContext(nc) as tc:
        with (
            tc.tile_pool(name="const", bufs=1) as const,
            tc.tile_pool(name="io", bufs=4) as io,
            tc.tile_pool(name="work", bufs=3) as work,
            tc.tile_pool(name="small", bufs=4) as small,
            tc.tile_pool(name="ps_qkv", bufs=cfg["qkv_b"],
                         space="PSUM") as ps_qkv,
            tc.tile_pool(name="ps_lg", bufs=cfg["lg_b"],
                         space="PSUM") as ps_lg,
            tc.tile_pool(name="ps_xa", bufs=cfg["xa_b"],
                         space="PSUM") as ps_xa,
            tc.tile_pool(name="ps_y", bufs=cfg["y_b"],
                         space="PSUM") as ps_y,
            tc.tile_pool(name="ps_xnT", bufs=cfg["xnT_b"],
                         space="PSUM") as ps_xnT,
        ):
            # ---- constants (bf16 prepared host-side) ----
            wqkv_sb = const.tile([P, 4, 3 * D], BF16)
            nc.gpsimd.dma_start(
                out=wqkv_sb, in_=wqkv.rearrange("(c p) n -> p c n", p=P))
            wo_sb = const.tile([P, 4, D], BF16)
            nc.gpsimd.dma_start(
                out=wo_sb, in_=wo_in.rearrange("(c p) n -> p c n", p=P))
            cos2_sb = const.tile([P, 2, D], BF16)
            nc.sync.dma_start(out=cos2_sb[:, 0, :], in_=cos_in)
            nc.sync.dma_start(out=cos2_sb[:, 1, :], in_=cos_in)
            ssin2_sb = const.tile([P, 2, D], BF16)
            nc.sync.dma_start(out=ssin2_sb[:, 0, :], in_=ssin_in)
            nc.sync.dma_start(out=ssin2_sb[:, 1, :], in_=ssin_in)
            ident_bf = const.tile([P, P], BF16)
            nc.gpsimd.dma_start(out=ident_bf, in_=ident_in)
            if with_bias_row:
                brow_sb = const.tile([1, 3 * D], BF16)
                nc.gpsimd.dma_start(out=brow_sb, in_=brow_in)
                ones_sb = const.tile([1, P], BF16)
                nc.gpsimd.dma_start(out=ones_sb, in_=ones_in)

            # persistent probs tiles, pre-zeroed once: exp/normalize only
            # touch valid blocks/rows, so cross-window slots stay 0 and
            # the K=128 PV contracts over an exact block-diagonal.
            nprobs = cfg["nprobs"]
            probs_tiles = []
            for z in range(nprobs):
                pz = work.tile([P, H, P], BF16, tag="probs",
                               name=f"pz{z}", bufs=nprobs)
                nc.gpsimd.memset(pz, 0.0)
                probs_tiles.append(pz)

            st = {}

            def stage_load(i):
                t0 = i * P
                x_t = io.tile([P, D], F32, tag="x", name=f"x{i}",
                              bufs=cfg["x_bufs"])
                nc.sync.dma_start(out=x_t, in_=x_in[t0:t0 + P, :])
                st[i] = {"x": x_t}

            def stage_ln(i):
                s = st[i]
                x_t = s["x"]
                stats = small.tile([P, 6], F32, tag="st", name=f"st{i}")
                nc.vector.bn_stats(out=stats, in_=x_t)
                mv = small.tile([P, 2], F32, tag="mv", name=f"mv{i}")
                nc.vector.bn_aggr(out=mv, in_=stats)
                # rsqrt(var+eps) via Newton on [P,1] tiles: y0=1.5-0.5v,
                # 2 iterations; v is within ~[0.6, 1.5] for unit-variance
                # rows so the result is accurate to ~1e-4.
                ne = eng("newton_on")
                v_t = small.tile([P, 1], F32, tag="vv", name=f"vv{i}")
                ne.tensor_scalar(
                    out=v_t, in0=mv[:, 1:2], scalar1=EPS, scalar2=None,
                    op0=ALU.add, op1=ALU.bypass)
                rstd = small.tile([P, 1], F32, tag="rs", name=f"rs{i}")
                ne.tensor_scalar(
                    out=rstd, in0=v_t, scalar1=-0.5, scalar2=1.5,
                    op0=ALU.mult, op1=ALU.add)
                tnv = small.tile([P, 1], F32, tag="tn", name=f"tn{i}")
                for _ in range(cfg["newton_iters"]):
                    ne.tensor_tensor(out=tnv, in0=rstd, in1=rstd,
                                     op=ALU.mult)
                    ne.tensor_tensor(out=tnv, in0=tnv, in1=v_t,
                                     op=ALU.mult)
                    ne.tensor_scalar(
                        out=tnv, in0=tnv, scalar1=-0.5, scalar2=1.5,
                        op0=ALU.mult, op1=ALU.add)
                    ne.tensor_tensor(out=rstd, in0=rstd, in1=tnv,
                                     op=ALU.mult)
                xn = work.tile([P, D], BF16, tag="xn", name=f"xn{i}",
                               bufs=6)
                if cfg["xn_on"] == "dve":
                    nc.vector.tensor_scalar(
                        out=xn, in0=x_t, scalar1=mv[:, 0:1], scalar2=rstd,
                        op0=ALU.subtract, op1=ALU.mult)
                else:
                    # xn = x*rstd + (-m*rstd) on ACT
                    nmr = small.tile([P, 1], F32, tag="nm", name=f"nm{i}")
                    ne.tensor_tensor(out=nmr, in0=mv[:, 0:1], in1=rstd,
                                     op=ALU.mult)
                    ne.tensor_scalar(out=nmr, in0=nmr, scalar1=-1.0,
                                     scalar2=None, op0=ALU.mult,
                                     op1=ALU.bypass)
                    nc.scalar.activation(out=xn, in_=x_t, func=ACTF.Copy,
                                         scale=rstd, bias=nmr)

                s["xn"] = xn

            def stage_xnT(i):
                s = st[i]
                xn = s.pop("xn")
                xnT_ps = ps_xnT.tile([P, 4, P], BF16, tag="xnTp",
                                     name=f"xnTp{i}")
                for c in range(4):
                    nc.tensor.transpose(
                        xnT_ps[:, c, :], xn[:, c * P:(c + 1) * P],
                        ident_bf)
                xnT = work.tile([P, 4, P], BF16, tag="xnT",
                                name=f"xnT{i}", bufs=6)
                ce = cfg["xnT_copy_on"]
                if ce == "act":
                    nc.scalar.copy(out=xnT, in_=xnT_ps)
                else:
                    eng("xnT_copy_on").tensor_copy(out=xnT, in_=xnT_ps)
                s["xnT"] = xnT

            def stage_qkv(i):
                s = st[i]
                xnT = s["xnT"]
                qkv_ps = ps_qkv.tile([P, 3, D], F32, tag="qkv",
                                     name=f"qkv{i}")
                for bk in range(3):
                    pt = qkv_ps[:, bk, :]
                    for c in range(4):
                        nc.tensor.matmul(
                            pt, xnT[:, c, :],
                            wqkv_sb[:, c, bk * D:(bk + 1) * D],
                            start=(c == 0),
                            stop=(c == 3 and not with_bias_row))
                    if with_bias_row:
                        nc.tensor.matmul(
                            pt, ones_sb, brow_sb[:, bk * D:(bk + 1) * D],
                            start=False, stop=True)
                s["qkv_ps"] = qkv_ps

            def stage_rope(i):
                s = st[i]
                qkv_ps = s.pop("qkv_ps")
                # ONE psum->SBUF bf16 copy for q,k,v (frees all 3 banks
                # with a single ordinal; v is consumed in place by PV)
                qkvc = work.tile([P, 3, D], BF16, tag="qkvc",
                                 name=f"qkvc{i}", bufs=10)
                if cfg["v_copy_on"] == "act":
                    nc.scalar.copy(out=qkvc, in_=qkv_ps)
                else:
                    nc.scalar.copy(out=qkvc[:, 0:2, :],
                                   in_=qkv_ps[:, 0:2, :])
                    eng("v_copy_on").tensor_copy(out=qkvc[:, 2, :],
                                                 in_=qkv_ps[:, 2, :])

                # RoPE on q and k in single wide ops
                qk2 = work.tile([P, 2, D], BF16, tag="qk2",
                                name=f"qk2{i}", bufs=6)
                tm2 = work.tile([P, 2, D], BF16, tag="tm2",
                                name=f"tm2_{i}")
                nc.vector.tensor_tensor(out=tm2, in0=qkvc[:, 0:2, :],
                                        in1=cos2_sb, op=ALU.mult)
                ts2 = work.tile([P, 2, D], BF16, tag="ts2",
                                name=f"ts2_{i}")
                qk_rot = _rot_view2(qkvc)
                nc.vector.tensor_tensor(
                    out=ts2.rearrange("p b (h t r) -> p b h t r", h=8,
                                      t=2),
                    in0=qk_rot,
                    in1=ssin2_sb.rearrange("p b (h t r) -> p b h t r",
                                           h=8, t=2),
                    op=ALU.mult)
                eng("adds_on").tensor_tensor(out=qk2, in0=tm2, in1=ts2,
                                             op=ALU.add)
                s["v"] = qkvc
                s["qk2"] = qk2

            def stage_qkT(i):
                s = st[i]
                qkT = work.tile([P, 2 * H, P], BF16, tag="qkT",
                                name=f"qkT{i}", bufs=6)
                nc.sync.dma_start_transpose(
                    out=qkT,
                    in_=s.pop("qk2").rearrange("p a b -> p (a b)"))
                s["qkT"] = qkT

            def stage_qk(i):
                s = st[i]
                qkT = s["qkT"]

                def qT_h(h):
                    return qkT[(h % 2) * 64:(h % 2) * 64 + 64, h // 2, :]

                def kT_h(h):
                    return qkT[(h % 2) * 64:(h % 2) * 64 + 64,
                               4 + h // 2, :]

                # QK per (head, window): all-valid logits in ONE psum
                # bank [128, 8, 64] (window w on partitions w*64..).
                lg = ps_lg.tile([P, H, 64], F32, tag="lg",
                                name=f"lg{i}")
                for h in range(H):
                    qT = qT_h(h)
                    kT = kT_h(h)
                    for w in range(2):
                        sl = slice(w * 64, w * 64 + 64)
                        nc.tensor.matmul(lg[sl, h, :], qT[:, sl],
                                         kT[:, sl], start=True,
                                         stop=True)
                s["lg"] = lg

            def stage_softmax(i):
                s = st[i]
                lg = s.pop("lg")
                probs = probs_tiles[i % nprobs]
                # dense exp of the valid diagonal blocks (all heads in
                # one op per window)
                nc.scalar.activation(
                    out=probs[0:64, :, 0:64], in_=lg[0:64, :, :],
                    func=ACTF.Exp, scale=0.125)
                nc.scalar.activation(
                    out=probs[64:128, :, 64:128], in_=lg[64:128, :, :],
                    func=ACTF.Exp, scale=0.125)
                # row sums over the zero-padded full rows (zeros add 0):
                # per-head in-place bypass TSPs with accum_out run in DVE
                # 4x mode, ~3x cheaper than one wide tensor_reduce
                sums = small.tile([P, H], F32, tag="sm", name=f"sm{i}")
                if cfg["sums_via"] == "tsp":
                    for h in range(H):
                        nc.vector.tensor_scalar(
                            out=probs[:, h, :], in0=probs[:, h, :],
                            scalar1=0.0, scalar2=None, op0=ALU.add,
                            op1=ALU.bypass, accum_out=sums[:, h:h + 1])
                else:
                    nc.vector.tensor_reduce(
                        out=sums, in_=probs, axis=mybir.AxisListType.X,
                        op=ALU.add)
                nc.vector.reciprocal(out=sums, in_=sums)
                for h in range(H):
                    nc.vector.tensor_scalar_mul(
                        out=probs[:, h, :], in0=probs[:, h, :],
                        scalar1=sums[:, h:h + 1])
                s["probs"] = probs

            def stage_pT(i):
                s = st[i]
                pT = work.tile([P, H, P], BF16, tag="pT", name=f"pT{i}",
                               bufs=6)
                nc.sync.dma_start_transpose(
                    out=pT,
                    in_=s.pop("probs").rearrange("p a b -> p (a b)"))
                s["pT"] = pT

            def stage_pv(i):
                s = st[i]
                pT = s["pT"]

                attnT_ps = ps_xa.tile([P, 4, P], F32, tag="xa",
                                      name=f"aTp{i}")
                for idx in range(H):
                    h = 2 * (idx % 4) + idx // 4
                    nc.tensor.matmul(
                        attnT_ps[(h % 2) * 64:(h % 2) * 64 + 64,
                                 h // 2, :],
                        s["v"][:, 2, h * 64:(h + 1) * 64], pT[:, h, :],
                        start=True, stop=True)
                attnT = work.tile([P, 4, P], BF16, tag="attnT",
                                  name=f"aT{i}", bufs=6)
                ce = cfg["attn_copy_on"]
                if ce == "act":
                    nc.scalar.copy(out=attnT, in_=attnT_ps)
                else:
                    eng("attn_copy_on").tensor_copy(out=attnT,
                                                    in_=attnT_ps)
                s["attnT"] = attnT
                if cfg["o_wo"] is None:
                    stage_wo(i)

            def stage_wo(i):
                s = st.pop(i)
                attnT = s["attnT"]
                y_ps = ps_y.tile([P, D], F32, tag="y", name=f"y{i}")
                for c in range(4):
                    nc.tensor.matmul(y_ps, attnT[:, c, :],
                                     wo_sb[:, c, :],
                                     start=(c == 0), stop=(c == 3))
                o_t = io.tile([P, D], F32, tag="o", name=f"o{i}")
                eng("resid_on").tensor_tensor(out=o_t, in0=y_ps,
                                              in1=s["x"], op=ALU.add)
                t0 = i * P
                nc.sync.dma_start(out=y_out[t0:t0 + P, :], in_=o_t)

            # (offset, stage) in per-iteration ISSUE order: work whose
            # deps completed in earlier iterations is issued first so
            # each in-order engine stream leads with ready work.
            by_name = {
                "load": (cfg["o_load"], stage_load),
                "ln": (cfg["o_ln"], stage_ln),
                "xnT": (cfg["o_xnT"], stage_xnT),
                "qkv": (cfg["o_qkv"], stage_qkv),
                "rope": (cfg["o_rope"], stage_rope),
                "qkT": (cfg["o_qkT"], stage_qkT),
                "qk": (cfg["o_qk"], stage_qk),
                "softmax": (cfg["o_softmax"], stage_softmax),
                "pT": (cfg["o_pT"], stage_pT),
                "pv": (cfg["o_pv"], stage_pv),
            }
            if cfg["o_wo"] is not None:
                by_name["wo"] = (cfg["o_wo"], stage_wo)
            stages = [by_name[n] for n in cfg["issue_order"]
                      if n in by_name]
            assert len(stages) == len(by_name)
            depth = max(o for o, _ in stages) + 1
            G = cfg["group"]
            n_groups = n_tiles // G
            for k in range(n_groups + depth - 1):
                for o, fn in stages:
                    g = k - o
                    if 0 <= g < n_groups:
                        for t in range(G * g, G * g + G):
                            fn(t)

    mk_shape = _marker_shape()
    nc.dram_tensor("uniq_marker", mk_shape, F32, kind="ExternalInput")
    nc.compile()
    return nc, tuple(mk_shape)


def _host_prep(inputs):
    x = np.asarray(inputs["x"], np.float32)
    ln_scale = np.asarray(inputs["ln_scale"], np.float32)
    ln_bias = np.asarray(inputs["ln_bias"], np.float32)
    wq = np.asarray(inputs["wq"], np.float32).reshape(D, D)
    wk = np.asarray(inputs["wk"], np.float32).reshape(D, D)
    wv = np.asarray(inputs["wv"], np.float32).reshape(D, D)
    wo = np.asarray(inputs["wo"], np.float32)
    windows = int(np.asarray(inputs["windows"]))
    assert windows == WINDOWS, f"unsupported windows={windows}"
    assert x.shape == (B, T, D)

    import ml_dtypes
    bf16 = ml_dtypes.bfloat16
    wcat = np.concatenate([wq, wk, wv], axis=1)  # [D, 3D]
    wqkv = np.ascontiguousarray(wcat * ln_scale[:, None]).astype(bf16)
    has_bias = bool(np.any(ln_bias != 0))
    brow = (ln_bias @ wcat).reshape(1, 3 * D).astype(bf16)

    n = T // windows  # 64
    inv = (1.0 / 10000.0 ** (np.arange(0, HD, 2, dtype=np.float64) / HD))
    pos = np.arange(n, dtype=np.float64)
    f = pos[:, None] * inv[None, :]  # [64, 32]
    ang = np.concatenate([f, f], axis=1)  # [64, 64]
    cos1 = np.cos(ang)
    sin1 = np.sin(ang)
    ssin1 = sin1.copy()
    ssin1[:, 0:32] *= -1.0  # sign folded: rot contribution
    cos_t = np.tile(np.tile(cos1, (2, 1)), (1, H)).astype(bf16)
    ssin_t = np.tile(np.tile(ssin1, (2, 1)), (1, H)).astype(bf16)
    return (x.reshape(B * T, D), wqkv, wo.astype(bf16), cos_t, ssin_t,
            has_bias, brow)


def kernel(**inputs):
    (xf, wqkv, wo, cos_t, ssin_t, has_bias, brow) = _host_prep(inputs)

    key = ("full", has_bias)
    if key not in _build_cache:
        _build_cache[key] = build(TOK_PER_CORE, with_bias_row=has_bias)
    nc, mk_shape = _build_cache[key]

    import ml_dtypes
    shared = {
        "wqkv": wqkv, "wo": np.ascontiguousarray(wo),
        "cos": cos_t, "ssin": ssin_t,
        "ident": np.eye(P, dtype=ml_dtypes.bfloat16),
        "uniq_marker": np.zeros(mk_shape, np.float32),
    }
    if has_bias:
        import ml_dtypes
        shared["brow"] = brow
        shared["onesrow"] = np.ones((1, P), ml_dtypes.bfloat16)
    in_maps = []
    for c in range(N_CORES):
        m = dict(shared)
        m["x"] = np.ascontiguousarray(
            xf[c * TOK_PER_CORE:(c + 1) * TOK_PER_CORE])
        in_maps.append(m)

    res = bass_utils.run_bass_kernel_spmd(
        nc, in_maps, core_ids=list(range(N_CORES)))
    out = np.concatenate([res.results[c]["y"] for c in range(N_CORES)],
                         axis=0)
    return out.reshape(B, T, D)
